# revision 5
# baseline (speedup 1.0000x reference)
"""Trainium2 Bass kernel for bag-level attention (ragged_sequence).

Math (per bag b over its 16 sentences i):
    att_i  = <x_i, rel[q_i]>
    w      = softmax(att) within bag
    logits = (sum_i w_i x_i) @ rel.T + bias

Key identity: logits[b] = sum_i w_i S[i,:] + bias with S = x @ rel.T, so x is
read from HBM exactly once. x and rel travel as fp16 (gate is rel_err<2e-2;
fp16 end-to-end lands ~1e-3). The contraction over D=768 is split into 6
chunks of 128; chunks 0-2 accumulate on PE column-tile (0,0) into PSUM rows
0:64, chunks 3-5 on tile (0,64) into rows 64:128, so the full score of class c
for sentence j is st[c,j] + st[75+c,j] (block-2 classes sit at rows 75:128,
leaving row 64 zero for the z-selector trick).

Device pipeline per 1024-sentence chunk (64 bags):
    st   = x @ rel.T on PE (12 fp16 matmuls, PSUM fp32)
    sc16 = st copied PSUM->SBUF fp16 by the ACT engine (single pass; every
           later DVE op then reads fp16 SBUF at 2x mode instead of PSUM 1x)
    sm   = sc16 * onehot(q)        (DVE)
    att  = ones128.T @ sm          (PE, two [1,512] matmuls)
    e    = exp(att) fp16           (ACT)
    ebs  = partition_broadcast(e)  (GpSimd)
    w    = (sc16 + sel64) * ebs    (DVE; row 64 of w is e, so its bag-sum = z)
    lu   = windowed reduce_16(w)   (DVE) -> [128, 64] fp32
    lc[:, 64-col slice] = sident.T @ lu   (PE) accumulated into a per-slab
           PSUM tile [64, 512]; sident folds block recombine + bias*z + a
           z-extraction column (row 53 of lc = z per bag)
Per 8-chunk slab: rz = 1/lc[53] (DVE), rzb broadcast (GpSimd),
logitsT = lc * rzb (DVE), DMA out via the ACT HWDGE ring.

Output is stored transposed [53, bags] and transposed back on host.
"""

import os
from contextlib import ExitStack

import ml_dtypes
import numpy as np

import concourse.bass as bass
import concourse.tile as tile
from concourse import bacc, library_config, mybir
from concourse.bass_utils import run_bass_kernel_spmd

# Problem constants (hardcoded per spec nn_Attention_85478439125349)
N = 262144
B = 16384
D = 768
C = 53
BAG = 16
N_CORES = 8
ROWS = N // N_CORES          # 32768 sentences per core
BAGS = B // N_CORES          # 2048 bags per core
KCH = D // 128               # 6 contraction chunks
F32 = mybir.dt.float32
F16 = mybir.dt.float16
F8 = mybir.dt.float8e4


def build_nc(rows: int, sc: int = 1024, ch: int = 1024, slab: int = 8) -> bass.Bass:
    """Build the per-core Bass program for `rows` sentences (bags of BAG)."""
    assert rows % sc == 0 and sc % ch == 0 and ch % BAG == 0
    bags = rows // BAG
    n_sc = rows // sc          # superchunks (DMA granularity)
    n_ch = sc // ch            # compute chunks per superchunk
    chb = ch // BAG            # bags per compute chunk (64)
    n_total = n_sc * n_ch
    assert n_total % slab == 0 and slab * chb <= 512

    nc = bacc.Bacc()
    # x fp16, partition-major packed per superchunk so each partition's
    # DMA run is KCH*sc contiguous elements: xt3[p, isc, k, j] =
    # xT[128k+p, isc*sc+j]
    xt3 = nc.declare_dram_parameter(
        "xt3", [128, rows // sc, KCH, sc], F16, isOutput=False
    )
    # one-hot mask replicated into both partition blocks: [128, rows], fp8
    oht = nc.declare_dram_parameter("oht", [128, rows], F8, isOutput=False)
    # relT packed for lhsT loads: relt[p, k, c] = rel[c, 128k+p], c pad to 64
    relt = nc.declare_dram_parameter("relt", [128, KCH, 64], F16, isOutput=False)
    # recombine matrix [128, 128] fp32: col c (<53) has 1.0 at rows c and 75+c
    # and bias_c at row 64; col 64 has 1.0 at row 64 (z extraction, placed at
    # 64 because engine APs may only start at partition 0/32/64/96); rest 0.
    sident = nc.declare_dram_parameter("sident", [128, 128], F32, isOutput=False)
    # selector column: 1.0 only in row 64 (puts e into w row 64 -> z in lu)
    sel64d = nc.declare_dram_parameter("sel64", [128, 1], F16, isOutput=False)
    out53 = nc.declare_dram_parameter("out53", [C, bags], F32, isOutput=True)

    with tile.TileContext(nc) as tc, ExitStack() as ctx:
        consts = ctx.enter_context(tc.tile_pool(name="consts", bufs=1))
        xpool = ctx.enter_context(tc.tile_pool(name="xpool", bufs=6))
        ohpool = ctx.enter_context(tc.tile_pool(name="ohpool", bufs=5))
        work = ctx.enter_context(tc.tile_pool(name="work", bufs=3))
        psum = ctx.enter_context(tc.tile_pool(name="psum", bufs=2, space="PSUM"))

        # --- constants ---
        relt_sb = consts.tile([128, KCH, 64], F16)
        nc.sync.dma_start(out=relt_sb, in_=relt[:, :, :])
        sident_sb = consts.tile([128, 128], F32)
        nc.sync.dma_start(out=sident_sb, in_=sident[:, :])
        ones128 = consts.tile([128, 1], F16)
        nc.vector.memset(ones128, 1.0)
        sel64 = consts.tile([128, 1], F16)
        nc.sync.dma_start(out=sel64, in_=sel64d[:, :])
        nc.gpsimd.load_library(library_config.attn)

        # Pipeline state: stage A (st/sc16/sm) at chunk i, stage B
        # (att/exp/ebs) at i-1, stage C (w/lu/lc + slab flush) at i-2.
        pend_a = {}   # i -> (sc16, sm)
        pend_b = {}   # i -> (sc16, ebs)
        lc_sl = [None]   # current slab PSUM tile [64, slab*chb]
        x_sb = oh_sb = None

        def stage_b(i):
            sc16, sm = pend_a.pop(i)
            e = work.tile([1, ch], F16, tag="e", bufs=3)
            for h in range(ch // 512):
                hs = slice(h * 512, h * 512 + 512)
                attp = psum.tile([1, 512], F32, tag="att", bufs=2)
                nc.tensor.matmul(attp, lhsT=ones128, rhs=sm[:, hs])
                nc.scalar.activation(
                    e[:, hs], attp, mybir.ActivationFunctionType.Exp
                )
            ebs = work.tile([128, ch], F16, tag="ebs", bufs=3)
            nc.gpsimd.partition_broadcast(ebs, e, channels=128)
            pend_b[i] = (sc16, ebs)

        def stage_c(i):
            sc16, ebs = pend_b.pop(i)
            w = work.tile([128, ch], F16, tag="w", bufs=2)
            nc.vector.scalar_tensor_tensor(
                w, sc16, sel64, ebs,
                op0=mybir.AluOpType.add, op1=mybir.AluOpType.mult,
            )
            lu = work.tile([128, chb], F32, tag="lu", bufs=3)
            nc.vector.reduce_sum(
                lu,
                w.rearrange("p (b j) -> p b j", j=BAG),
                axis=mybir.AxisListType.X,
            )
            if i % slab == 0:
                lc_sl[0] = psum.tile(
                    [128, slab * chb], F32, tag="lc", bufs=2, name="lc"
                )
            ob = (i % slab) * chb
            # recombines the two partition blocks, folds bias*z into rows
            # 0:53 and extracts z into row 53
            nc.tensor.matmul(
                lc_sl[0][:, ob : ob + chb], lhsT=sident_sb, rhs=lu
            )
            if (i + 1) % slab == 0:
                lc = lc_sl[0]
                nb = slab * chb
                rz = work.tile([1, nb], F32, tag="rz", bufs=2)
                nc.vector.reciprocal(rz, lc[64:65, :])
                rzb = work.tile([C, nb], F32, tag="rzb", bufs=2)
                nc.gpsimd.partition_broadcast(rzb, rz, channels=C)
                lt = work.tile([C, nb], F32, tag="lt", bufs=2)
                nc.vector.tensor_mul(lt, lc[0:C, :], rzb)
                s0 = (i + 1 - slab) * chb
                # out DMA issued from the ACT HWDGE ring so it never blocks
                # the sync engine's x-prefetch issues
                nc.scalar.dma_start(out=out53[:, s0 : s0 + nb], in_=lt)

        for i in range(n_total + 2):
            if i < n_total:
                isc, ic = divmod(i, n_ch)
                if ic == 0:
                    x_sb = xpool.tile([128, KCH, sc], F16, bufs=6)
                    nc.sync.dma_start(out=x_sb, in_=xt3[:, isc, :, :])
                    oh_sb = ohpool.tile([128, sc], F8, bufs=5)
                    nc.sync.dma_start(
                        out=oh_sb, in_=oht[:, isc * sc : (isc + 1) * sc]
                    )
                cs = slice(ic * ch, (ic + 1) * ch)
                st = psum.tile([128, ch], F32, tag="st", bufs=2)
                # Each column-half runs its own start=True accumulation chain
                # ((0,64) emitted first). matmul moving dim is ISA-capped at
                # 512 columns, so each k-chunk is fed in 512-wide halves.
                for h in range(ch // 512):
                    hs = slice(ic * ch + h * 512, ic * ch + h * 512 + 512)
                    os_ = slice(h * 512, h * 512 + 512)
                    for k in range(KCH // 2, KCH):
                        nc.tensor.matmul(
                            st[64:128, os_],
                            lhsT=relt_sb[:, k, :],
                            rhs=x_sb[:, k, hs],
                            start=(k == KCH // 2),
                            stop=False,
                            skip_group_check=True,
                            tile_position=(0, 64),
                        )
                    for k in range(KCH // 2):
                        nc.tensor.matmul(
                            st[0:64, os_],
                            lhsT=relt_sb[:, k, :],
                            rhs=x_sb[:, k, hs],
                            start=(k == 0),
                            stop=(k == KCH // 2 - 1),
                            skip_group_check=True,
                            tile_position=(0, 0),
                        )
                # one PSUM->SBUF fp16 pass on the (otherwise idle) ACT
                # engine; all later elementwise ops read SBUF fp16 at 2x
                sc16 = work.tile([128, ch], F16, tag="sc16", bufs=4)
                nc.scalar.copy(sc16, st)
            # stage C before sm: its inputs are 2 chunks old (surely ready),
            # so DVE starts the iteration with runnable work instead of
            # blocking on the ACT copy of this chunk
            if 0 <= i - 2 < n_total:
                stage_c(i - 2)
            if i < n_total:
                sm = work.tile([128, ch], F16, tag="sm", bufs=3)
                nc.vector.tensor_mul(sm, sc16, oh_sb[:, cs])
                pend_a[i] = (sc16, sm)
            if 0 <= i - 1 < n_total:
                stage_b(i - 1)
    return nc


_NC_CACHE: dict = {}


def _get_nc(rows: int) -> bass.Bass:
    if rows not in _NC_CACHE:
        nc = build_nc(rows)
        nc.finalize()
        _NC_CACHE[rows] = nc
    return _NC_CACHE[rows]


def _numpy_fallback(x, rel_weight, bias, input_scope, query):
    """Pure-numpy replication of the reference for non-uniform bag layouts."""
    n = x.shape[0]
    num_bags = input_scope.shape[0] - 1
    seg = np.searchsorted(input_scope[1:], np.arange(n), side="right")
    att = np.einsum("nd,nd->n", x, rel_weight[query]).astype(np.float32)
    valid = seg < num_bags
    segv = seg[valid]
    attv = att[valid]
    m = np.full(num_bags, -np.inf, dtype=np.float32)
    np.maximum.at(m, segv, attv)
    e = np.zeros(n, dtype=np.float32)
    e[valid] = np.exp(attv - m[segv])
    z = np.zeros(num_bags, dtype=np.float32)
    np.add.at(z, segv, e[valid])
    w = np.zeros(n, dtype=np.float32)
    nz = z[segv] != 0
    w_valid = np.zeros(segv.shape[0], dtype=np.float32)
    w_valid[nz] = e[valid][nz] / z[segv][nz]
    w[valid] = w_valid
    repre = np.zeros((num_bags, x.shape[1]), dtype=np.float32)
    np.add.at(repre, segv, (x[valid] * w[valid][:, None]).astype(np.float32))
    return repre @ rel_weight.T + bias


def _pack_x(x_core, sc):
    """[rows, D] fp32 -> [128, rows//sc, KCH, sc] fp16 so each partition's
    per-superchunk DMA run (KCH*sc elements) is contiguous."""
    rows = x_core.shape[0]
    xt = x_core.astype(np.float16).T                     # [D, rows]
    v = np.ascontiguousarray(xt).reshape(KCH, 128, rows // sc, sc)
    return np.ascontiguousarray(v.transpose(1, 2, 0, 3))


def _prepare_in_maps(x, rel_weight, bias, query, sc=1024):
    # block-1 (k=0..2) classes in columns 0:53 -> st rows 0:53;
    # block-2 (k=3..5) classes in columns 11:64 -> st rows 75:128, leaving
    # st row 64 zero for the z-selector trick
    rt = rel_weight.astype(np.float16).T.reshape(KCH, 128, C).transpose(1, 0, 2)
    relt = np.zeros((128, KCH, 64), dtype=np.float16)
    relt[:, : KCH // 2, :C] = rt[:, : KCH // 2, :]
    relt[:, KCH // 2 :, 11 : 11 + C] = rt[:, KCH // 2 :, :]
    sident = np.zeros((128, 128), dtype=np.float32)
    sident[np.arange(C), np.arange(C)] = 1.0
    sident[75 + np.arange(C), np.arange(C)] = 1.0
    # row 64 of lu is z, so a bias row folds bias*z into the recombine and
    # column 64 extracts z itself (for the batched reciprocal)
    sident[64, :C] = bias.astype(np.float32)
    sident[64, 64] = 1.0
    sel64 = np.zeros((128, 1), dtype=np.float16)
    sel64[64, 0] = 1.0
    q = query.astype(np.int64)
    in_maps = []
    for c in range(N_CORES):
        lo_r, hi_r = c * ROWS, (c + 1) * ROWS
        # fp8e4m3 one-hot built via its bit pattern (1.0 == 0x38)
        oh8 = np.zeros((128, ROWS), dtype=np.uint8)
        qc = q[lo_r:hi_r]
        ar = np.arange(ROWS)
        oh8[qc, ar] = 0x38
        oh8[75 + qc, ar] = 0x38
        oh = oh8.view(ml_dtypes.float8_e4m3)
        in_maps.append(
            {"xt3": _pack_x(x[lo_r:hi_r], sc), "oht": oh,
             "relt": relt, "sident": sident, "sel64": sel64}
        )
    return in_maps


def run_device(x, rel_weight, bias, query, trace=False, **kwargs):
    nc = _get_nc(ROWS)
    in_maps = _prepare_in_maps(x, rel_weight, bias, query)
    res = run_bass_kernel_spmd(
        nc, in_maps, core_ids=list(range(N_CORES)), trace=trace, **kwargs
    )
    outs = [np.ascontiguousarray(np.asarray(r["out53"]).T) for r in res.results]
    return np.concatenate(outs, axis=0), res


def kernel(x, rel_weight, bias, input_scope, query):
    x = np.asarray(x, dtype=np.float32)
    rel_weight = np.asarray(rel_weight, dtype=np.float32)
    bias = np.asarray(bias, dtype=np.float32)
    input_scope = np.asarray(input_scope)
    query = np.asarray(query)

    expected_scope = np.arange(B + 1, dtype=np.int64) * (N // B)
    if (
        x.shape == (N, D)
        and rel_weight.shape == (C, D)
        and input_scope.shape == (B + 1,)
        and np.array_equal(input_scope.astype(np.int64), expected_scope)
    ):
        out, _ = run_device(x, rel_weight, bias, query)
        return out
    return _numpy_fallback(x, rel_weight, bias, input_scope, query)


# revision 8
# speedup vs baseline: 1.4206x; 1.4206x over previous
"""Trainium2 Bass kernel for bag-level attention (ragged_sequence).

Math (per bag b over its 16 sentences i):
    att_i  = <x_i, rel[q_i]>
    w      = softmax(att) within bag
    logits = (sum_i w_i x_i) @ rel.T + bias

Key identity: logits[b] = sum_i w_i S[i,:] + bias with S = x @ rel.T, so x is
read from HBM exactly once. x and rel travel as fp16 (gate is rel_err<2e-2;
fp16 end-to-end lands ~1e-3). The contraction over D=768 is split into 6
chunks of 128; chunks 0-2 accumulate on PE column-tile (0,0) into PSUM rows
0:64, chunks 3-5 on tile (0,64) into rows 64:128, so the full score of class c
for sentence j is st[c,j] + st[75+c,j] (block-2 classes sit at rows 75:128,
leaving st row 64 zero for the z-selector trick).

Device pipeline per 1024-sentence chunk (64 bags), software-pipelined with a
DEEP skew — stage A at chunk i, stage B at i-2, stage C at i-4 — so every
engine always has runnable work and the softmax latency chain (5 engine hops)
never gates the next chunk's matmuls:
  A:  st   = x @ rel.T              (PE, 12 fp16 matmuls, PSUM fp32)
      sc16 = (st + sel64) fp16      (ACT copy PSUM->SBUF with sel64 as the
             free per-partition bias; this copy is what breaks the st-PSUM
             lifetime so st needs only 2 PSUM bufs despite the deep skew)
      sm   = st * onehot(q)         (DVE, PSUM fp32 x fp8 -> SBUF fp16)
  B:  att  = ones128.T @ sm         (PE, two [1,512] matmuls)
      e    = exp(att - 4*ln2)       (ACT; the -4ln2 bias scales e by 1/16 so
             the weighted sums below stay inside fp16 range; it divides out
             of the final softmax normalize exactly)
      ebs  = partition_broadcast(e) (GpSimd, two 512 halves)
  C:  w    = sc16 * ebs             (DVE, fp16 SBUF; row 64 of w is e)
      lu   = windowed reduce_16(w)  (DVE) -> [128, 64] fp16
      lc[:, 64-col slice] = sident.T @ lu  (PE) accumulated into a per-slab
             PSUM tile [128, 512]; sident folds block recombine + bias*z and
             extracts z into row 64
Per 8-chunk slab: ACT copies lc rows 0:65 PSUM->SBUF and DMAs them out via
the ACT HWDGE ring. The final divide by z (row 64) happens on the host.

Output is stored transposed [65, bags]; host divides and transposes back.
"""

import os
from contextlib import ExitStack

import ml_dtypes
import numpy as np

import concourse.bass as bass
import concourse.tile as tile
from concourse import bacc, library_config, mybir
from concourse.bass_utils import run_bass_kernel_spmd

# Problem constants (hardcoded per spec nn_Attention_85478439125349)
N = 262144
B = 16384
D = 768
C = 53
BAG = 16
N_CORES = 8
ROWS = N // N_CORES          # 32768 sentences per core
BAGS = B // N_CORES          # 2048 bags per core
KCH = D // 128               # 6 contraction chunks
F32 = mybir.dt.float32
F16 = mybir.dt.float16
F8 = mybir.dt.float8e4
EXP_BIAS = -2.772588722239781   # -4*ln2: e' = e/16, cancels in w = e'/z'


def build_nc(rows: int, sc: int = 1024, ch: int = 1024, slab: int = 8) -> bass.Bass:
    """Build the per-core Bass program for `rows` sentences (bags of BAG)."""
    assert rows % sc == 0 and sc % ch == 0 and ch % BAG == 0
    bags = rows // BAG
    n_sc = rows // sc          # superchunks (DMA granularity)
    n_ch = sc // ch            # compute chunks per superchunk
    chb = ch // BAG            # bags per compute chunk (64)
    n_total = n_sc * n_ch
    assert n_total % slab == 0 and slab * chb <= 512

    nc = bacc.Bacc()
    # x fp16, partition-major packed per superchunk so each partition's
    # DMA run is KCH*sc contiguous elements: xt3[p, isc, k, j] =
    # xT[128k+p, isc*sc+j]
    xt3 = nc.declare_dram_parameter(
        "xt3", [128, rows // sc, KCH, sc], F16, isOutput=False
    )
    # one-hot mask replicated into both partition blocks: [128, rows], fp8
    oht = nc.declare_dram_parameter("oht", [128, rows], F8, isOutput=False)
    # relT packed for lhsT loads: relt[p, k, c] = rel[c, 128k+p], c pad to 64
    relt = nc.declare_dram_parameter("relt", [128, KCH, 64], F16, isOutput=False)
    # recombine matrix [128, 128] fp16: col c (<53) has 1.0 at rows c and 75+c
    # and bias_c at row 64; col 64 has 1.0 at row 64 (z extraction, placed at
    # 64 because engine APs may only start at partition 0/32/64/96); rest 0.
    sident = nc.declare_dram_parameter("sident", [128, 128], F16, isOutput=False)
    # selector column: 1.0 only in row 64 (puts e into w row 64 -> z in lu)
    sel64d = nc.declare_dram_parameter("sel64", [128, 1], F16, isOutput=False)
    # un-normalized logits (rows 0:53) + z (row 64), divided on host
    out65 = nc.declare_dram_parameter("out65", [65, bags], F32, isOutput=True)

    with tile.TileContext(nc) as tc, ExitStack() as ctx:
        consts = ctx.enter_context(tc.tile_pool(name="consts", bufs=1))
        xpool = ctx.enter_context(tc.tile_pool(name="xpool", bufs=6))
        ohpool = ctx.enter_context(tc.tile_pool(name="ohpool", bufs=5))
        work = ctx.enter_context(tc.tile_pool(name="work", bufs=3))
        psum = ctx.enter_context(tc.tile_pool(name="psum", bufs=2, space="PSUM"))

        # --- constants ---
        relt_sb = consts.tile([128, KCH, 64], F16)
        nc.sync.dma_start(out=relt_sb, in_=relt[:, :, :])
        sident_sb = consts.tile([128, 128], F16)
        nc.sync.dma_start(out=sident_sb, in_=sident[:, :])
        ones128 = consts.tile([128, 1], F16)
        nc.vector.memset(ones128, 1.0)
        sel64 = consts.tile([128, 1], F16)
        nc.sync.dma_start(out=sel64, in_=sel64d[:, :])
        ebias = consts.tile([128, 1], F32)
        nc.vector.memset(ebias, EXP_BIAS)
        nc.gpsimd.load_library(library_config.attn)

        pend_a = {}   # i -> (sc16, sm)
        pend_b = {}   # i -> (sc16, ebs)
        lc_sl = [None]   # current slab PSUM tile [128, slab*chb]
        x_sb = oh_sb = None

        def stage_b(i):
            sc16, sm = pend_a.pop(i)
            e = work.tile([1, ch], F16, tag="e", bufs=3)
            ebs = work.tile([128, ch], F16, tag="ebs", bufs=4)
            for h in range(ch // 512):
                hs = slice(h * 512, h * 512 + 512)
                attp = psum.tile([1, 512], F32, tag="att", bufs=2)
                nc.tensor.matmul(attp, lhsT=ones128, rhs=sm[:, hs])
                nc.scalar.activation(
                    e[:, hs], attp, mybir.ActivationFunctionType.Exp,
                    bias=ebias[0:1, :],
                )
                nc.gpsimd.partition_broadcast(ebs[:, hs], e[:, hs], channels=128)
            pend_b[i] = (sc16, ebs)

        def stage_c(i):
            sc16, ebs = pend_b.pop(i)
            w = work.tile([128, ch], F16, tag="w", bufs=2)
            nc.vector.tensor_mul(w, sc16, ebs)
            lu = work.tile([128, chb], F16, tag="lu", bufs=3)
            with nc.allow_low_precision("fp16 bag sums stay < 2^14, rel 5e-4"):
                nc.vector.reduce_sum(
                    lu,
                    w.rearrange("p (b j) -> p b j", j=BAG),
                    axis=mybir.AxisListType.X,
                )
            if i % slab == 0:
                lc_sl[0] = psum.tile(
                    [128, slab * chb], F32, tag="lc", bufs=2, name="lc"
                )
            ob = (i % slab) * chb
            # recombines the two partition blocks, folds bias*z into rows
            # 0:53 and extracts z into row 64
            nc.tensor.matmul(
                lc_sl[0][:, ob : ob + chb], lhsT=sident_sb, rhs=lu
            )
            if (i + 1) % slab == 0:
                nb = slab * chb
                ltc = work.tile([65, nb], F32, tag="ltc", bufs=2)
                nc.scalar.copy(ltc, lc_sl[0][0:65, :])
                s0 = (i + 1 - slab) * chb
                # out DMA issued from the ACT HWDGE ring so it never blocks
                # the sync engine's x-prefetch issues
                nc.scalar.dma_start(out=out65[:, s0 : s0 + nb], in_=ltc)

        for i in range(n_total + 4):
            if i < n_total:
                isc, ic = divmod(i, n_ch)
                if ic == 0:
                    x_sb = xpool.tile([128, KCH, sc], F16, bufs=6)
                    nc.sync.dma_start(out=x_sb, in_=xt3[:, isc, :, :])
                    oh_sb = ohpool.tile([128, sc], F8, bufs=5)
                    nc.sync.dma_start(
                        out=oh_sb, in_=oht[:, isc * sc : (isc + 1) * sc]
                    )
                cs = slice(ic * ch, (ic + 1) * ch)
                st = psum.tile([128, ch], F32, tag="st", bufs=2)
                # Each column-half runs its own start=True accumulation chain
                # ((0,64) emitted first). matmul moving dim is ISA-capped at
                # 512 columns, so each k-chunk is fed in 512-wide halves.
                for h in range(ch // 512):
                    hs = slice(ic * ch + h * 512, ic * ch + h * 512 + 512)
                    os_ = slice(h * 512, h * 512 + 512)
                    for k in range(KCH // 2, KCH):
                        nc.tensor.matmul(
                            st[64:128, os_],
                            lhsT=relt_sb[:, k, :],
                            rhs=x_sb[:, k, hs],
                            start=(k == KCH // 2),
                            stop=False,
                            skip_group_check=True,
                            tile_position=(0, 64),
                        )
                    for k in range(KCH // 2):
                        nc.tensor.matmul(
                            st[0:64, os_],
                            lhsT=relt_sb[:, k, :],
                            rhs=x_sb[:, k, hs],
                            start=(k == 0),
                            stop=(k == KCH // 2 - 1),
                            skip_group_check=True,
                            tile_position=(0, 0),
                        )
                # one PSUM->SBUF fp16 pass on the (otherwise idle) ACT
                # engine, folding the +sel64 in as the free per-partition
                # bias; releases st after this iteration (st bufs=2) even
                # though w consumes the scores 4 chunks later
                sc16 = work.tile([128, ch], F16, tag="sc16", bufs=6)
                nc.scalar.activation(
                    sc16, st, mybir.ActivationFunctionType.Identity, bias=sel64
                )
            # stage C first: its inputs are 4 chunks old (surely ready), so
            # DVE opens the iteration with runnable work instead of blocking
            # on this chunk's matmuls
            if 0 <= i - 4 < n_total:
                stage_c(i - 4)
            if i < n_total:
                sm = work.tile([128, ch], F16, tag="sm", bufs=4)
                nc.vector.tensor_mul(sm, st, oh_sb[:, cs])
                pend_a[i] = (sc16, sm)
            if 0 <= i - 2 < n_total:
                stage_b(i - 2)
    return nc


_NC_CACHE: dict = {}


def _get_nc(rows: int) -> bass.Bass:
    if rows not in _NC_CACHE:
        nc = build_nc(rows)
        nc.finalize()
        _NC_CACHE[rows] = nc
    return _NC_CACHE[rows]


def _numpy_fallback(x, rel_weight, bias, input_scope, query):
    """Pure-numpy replication of the reference for non-uniform bag layouts."""
    n = x.shape[0]
    num_bags = input_scope.shape[0] - 1
    seg = np.searchsorted(input_scope[1:], np.arange(n), side="right")
    att = np.einsum("nd,nd->n", x, rel_weight[query]).astype(np.float32)
    valid = seg < num_bags
    segv = seg[valid]
    attv = att[valid]
    m = np.full(num_bags, -np.inf, dtype=np.float32)
    np.maximum.at(m, segv, attv)
    e = np.zeros(n, dtype=np.float32)
    e[valid] = np.exp(attv - m[segv])
    z = np.zeros(num_bags, dtype=np.float32)
    np.add.at(z, segv, e[valid])
    w = np.zeros(n, dtype=np.float32)
    nz = z[segv] != 0
    w_valid = np.zeros(segv.shape[0], dtype=np.float32)
    w_valid[nz] = e[valid][nz] / z[segv][nz]
    w[valid] = w_valid
    repre = np.zeros((num_bags, x.shape[1]), dtype=np.float32)
    np.add.at(repre, segv, (x[valid] * w[valid][:, None]).astype(np.float32))
    return repre @ rel_weight.T + bias


def _pack_x(x_core, sc):
    """[rows, D] fp32 -> [128, rows//sc, KCH, sc] fp16 so each partition's
    per-superchunk DMA run (KCH*sc elements) is contiguous."""
    rows = x_core.shape[0]
    xt = x_core.astype(np.float16).T                     # [D, rows]
    v = np.ascontiguousarray(xt).reshape(KCH, 128, rows // sc, sc)
    return np.ascontiguousarray(v.transpose(1, 2, 0, 3))


def _prepare_in_maps(x, rel_weight, bias, query, sc=1024):
    # block-1 (k=0..2) classes in columns 0:53 -> st rows 0:53;
    # block-2 (k=3..5) classes in columns 11:64 -> st rows 75:128, leaving
    # st row 64 zero for the z-selector trick
    rt = rel_weight.astype(np.float16).T.reshape(KCH, 128, C).transpose(1, 0, 2)
    relt = np.zeros((128, KCH, 64), dtype=np.float16)
    relt[:, : KCH // 2, :C] = rt[:, : KCH // 2, :]
    relt[:, KCH // 2 :, 11 : 11 + C] = rt[:, KCH // 2 :, :]
    sident = np.zeros((128, 128), dtype=np.float16)
    sident[np.arange(C), np.arange(C)] = 1.0
    sident[75 + np.arange(C), np.arange(C)] = 1.0
    # row 64 of lu is z, so a bias row folds bias*z into the recombine and
    # column 64 extracts z itself (for the host-side divide)
    sident[64, :C] = bias.astype(np.float16)
    sident[64, 64] = 1.0
    sel64 = np.zeros((128, 1), dtype=np.float16)
    sel64[64, 0] = 1.0
    q = query.astype(np.int64)
    in_maps = []
    for c in range(N_CORES):
        lo_r, hi_r = c * ROWS, (c + 1) * ROWS
        # fp8e4m3 one-hot built via its bit pattern (1.0 == 0x38)
        oh8 = np.zeros((128, ROWS), dtype=np.uint8)
        qc = q[lo_r:hi_r]
        ar = np.arange(ROWS)
        oh8[qc, ar] = 0x38
        oh8[75 + qc, ar] = 0x38
        oh = oh8.view(ml_dtypes.float8_e4m3)
        in_maps.append(
            {"xt3": _pack_x(x[lo_r:hi_r], sc), "oht": oh,
             "relt": relt, "sident": sident, "sel64": sel64}
        )
    return in_maps


def run_device(x, rel_weight, bias, query, trace=False, **kwargs):
    nc = _get_nc(ROWS)
    in_maps = _prepare_in_maps(x, rel_weight, bias, query)
    res = run_bass_kernel_spmd(
        nc, in_maps, core_ids=list(range(N_CORES)), trace=trace, **kwargs
    )
    outs = []
    for r in res.results:
        lt = np.asarray(r["out65"]).astype(np.float64)
        logits = (lt[0:C, :] / lt[64:65, :]).T.astype(np.float32)
        outs.append(np.ascontiguousarray(logits))
    return np.concatenate(outs, axis=0), res


def kernel(x, rel_weight, bias, input_scope, query):
    x = np.asarray(x, dtype=np.float32)
    rel_weight = np.asarray(rel_weight, dtype=np.float32)
    bias = np.asarray(bias, dtype=np.float32)
    input_scope = np.asarray(input_scope)
    query = np.asarray(query)

    expected_scope = np.arange(B + 1, dtype=np.int64) * (N // B)
    if (
        x.shape == (N, D)
        and rel_weight.shape == (C, D)
        and input_scope.shape == (B + 1,)
        and np.array_equal(input_scope.astype(np.int64), expected_scope)
    ):
        out, _ = run_device(x, rel_weight, bias, query)
        return out
    return _numpy_fallback(x, rel_weight, bias, input_scope, query)
